# revision 51
# baseline (speedup 1.0000x reference)
"""Chunked (= full, non-causal) multi-head self-attention on 8 TRN2 NeuronCores.

Problem: B=2, S=2048, D=1024, H=16 heads (head_dim 64), torch-Linear-style
projections (y = x @ W.T + b), softmax attention, output projection.

Sharding: head-parallel. Core c owns heads {2c, 2c+1} = feature slice
[128c, 128c+128). Each core computes q/k/v for its slice from the full x
(replicated), runs attention for its 4 (batch, head) pairs, and produces a
partial output projection with its 128-row slice of Wo. Host sums the 8
partials and adds bo.

Layout: scores are computed transposed, ST[k, q] (keys on partitions), so the
softmax exp output PT feeds the P@V matmul directly (contraction over k on
partitions) with no on-chip transposes anywhere — x and the weights are
pre-transposed on the host. The two heads' K=64 score matmuls land on PE
row-groups 0-1/2-3. The softmax denominator rides as row 64 of the PV output
via a ones-column appended to V (M=65); both heads' PV accumulators share one
2-bank PSUM tile so the iteration epilogue (raw-o copy, reciprocal via
exp(-ln) on ACT, GPSIMD partition-broadcast, normalize-multiply) runs as wide
1024-col ops, deferred one iteration so it never stalls the PE.

Softmax exp is split across two engines: the ACT (scalar) engine computes the
table-based exp for ~1/3 of the score tiles, and the DVE (vector) engine
computes the rest with a one-instruction Schraudolph exp — bf16 bits =
u16(x * 128*log2(e)/sqrt(hd) + (127*128 - 5.51)), i.e. a tensor_scalar mul+add
with u16-saturating convert whose output is bitcast to bf16 (underflow
saturates to +0.0 exactly). The per-element error (~±3%, ~1.8% mean) averages
out below 2e-3 in the softmax-weighted sum, and each head-tile's denominator
rides the same approximated values, so normalization stays self-consistent.

Scheduling: one software-pipelined stream keeps the PE densely busy (TRN2's
HAM clock-gate halves the PE clock after ~3.4us of idle): x-strip DMAs issue
on the Sync queue ahead of the weight DMAs (Scalar queue), batch-0 k/v (+q0)
projections run first, then batch-0 attention interleaved with the remaining
projections, then batch-1 attention interleaved with the deferred output
projections. y tiles evict PSUM->SBUF alternating between DVE and ACT and
stream out as bf16 (host accumulates in fp64).

Precision: everything bf16 in, fp32 accumulate, bf16 partial outputs
(~6e-3 rel err overall).
"""

import sys

if "/opt/trn_rl_repo" not in sys.path:
    sys.path.insert(0, "/opt/trn_rl_repo")

import numpy as np

import concourse.bacc as bacc
import concourse.mybir as mybir
import concourse.tile as tile
from concourse import bass_utils

# Route Exp to the activation-table set that also holds Ln, so the softmax
# exps and the reciprocal-via-exp(-ln(x)) trick share one table (the default
# per-function choice would ping-pong table loads at ~2.7us each).
_orig_get_activation_tables = bacc.get_activation_tables


def _patched_get_activation_tables(arch):
    out = {}
    for name, funcs in dict(_orig_get_activation_tables(arch)).items():
        if name != "natural_log_exp_and_others":
            funcs = {f for f in funcs if f != mybir.ActivationFunctionType.Exp}
        out[name] = funcs
    return out


bacc.get_activation_tables = _patched_get_activation_tables

B, S, D, H = 2, 2048, 1024, 16
HD = D // H          # 64
NCORES = 8
ES = D // NCORES     # 128 features (= 2 heads) per core
BS = B * S           # 4096 rows total

P = 128              # partitions
NF = 512             # matmul free-dim tile
N_SB = BS // NF      # 8 s-blocks of 512
N_DC = D // P        # 8 contraction chunks of 128
N_KB = S // P        # 16 key blocks of 128 per batch
N_KP = N_KB // 2     # 8 key-block PAIRS per batch
N_QC = S // NF       # 4 query chunks of 512 per batch
N_CH = BS // P       # 32 global 128-row chunks

F32 = mybir.dt.float32
BF16 = mybir.dt.bfloat16
U16 = mybir.dt.uint16

DT_QK = BF16         # x/Wq/Wk inputs for q,k projections + score matmuls
DT_V = BF16          # x/Wv inputs for v projection
DT_ATT = BF16        # attention weights (exp output) and V in the P@V matmul
DT_OUT = BF16        # output projection inputs (OT, Wo)
DT_Y = BF16          # y partials shipped to the host

# Schraudolph-to-bf16 exp constants: bf16 bits = u16(x*A16 + B16)
LOG2E = float(np.log2(np.e))
A16 = LOG2E * 128.0
B16 = 127.0 * 128.0 - 128.0 * 0.04305

_cache = {}
last_results = None          # test.py reads exec_time_ns off this


def _np_dt(dt):
    import ml_dtypes

    return np.dtype(ml_dtypes.bfloat16) if dt == mybir.dt.bfloat16 else np.dtype(np.float32)


def _build():
    nc = bacc.Bacc("TRN2", target_bir_lowering=False, debug=False)

    xT_d = nc.dram_tensor("xT", [D, BS], DT_QK, kind="ExternalInput")
    # weights arrive host-pre-rearranged to the SBUF layout [P, N_DC, ES]
    # (dense per-partition rows -> trivial DMA descriptors, ~5x faster issue)
    wqT_d = nc.dram_tensor("wqT", [P, N_DC, ES], DT_QK, kind="ExternalInput")
    wkT_d = nc.dram_tensor("wkT", [P, N_DC, ES], DT_QK, kind="ExternalInput")
    wvT_d = nc.dram_tensor("wvT", [P, N_DC, ES], DT_V, kind="ExternalInput")
    bq_d = nc.dram_tensor("bq", [ES, 1], F32, kind="ExternalInput")
    bk_d = nc.dram_tensor("bk", [ES, 1], F32, kind="ExternalInput")
    bv_d = nc.dram_tensor("bv", [1, ES], F32, kind="ExternalInput")
    woT_d = nc.dram_tensor("woT", [ES, D], DT_OUT, kind="ExternalInput")
    y_d = nc.dram_tensor("y", [BS, D], DT_Y, kind="ExternalOutput")

    inv_sqrt_hd = 1.0 / float(np.sqrt(HD))

    with tile.TileContext(nc) as tc:
        with tc.tile_pool(name="const", bufs=1) as cpool, \
             tc.tile_pool(name="xt", bufs=3) as xt_pool, \
             tc.tile_pool(name="qkv", bufs=1) as qkv_pool, \
             tc.tile_pool(name="pt", bufs=14) as pt_pool, \
             tc.tile_pool(name="ysb", bufs=6) as y_pool, \
             tc.tile_pool(name="ps", bufs=1, space="PSUM") as ps:

            # ---- input DMAs: x strips on Sync, weights on Scalar ---------
            strips = {}

            xT_r = xT_d.ap().rearrange("(a p) s -> p a s", p=P)

            def emit_strip_dma(sb, split=False):
                strip = xt_pool.tile([P, N_DC, NF], DT_QK, tag="strip", name=f"strip{sb}")
                if split:
                    # two half-DMAs: the first qk matmuls (contraction chunks
                    # 0-3) start after only half the strip has landed
                    sl = xT_r[:, :, sb * NF : (sb + 1) * NF]
                    nc.sync.dma_start(strip[:, 0 : N_DC // 2], sl[:, 0 : N_DC // 2])
                    nc.sync.dma_start(strip[:, N_DC // 2 :], sl[:, N_DC // 2 :])
                else:
                    nc.sync.dma_start(strip[:], xT_r[:, :, sb * NF : (sb + 1) * NF])
                strips[sb] = strip

            # minimal DMA set before the first matmul: strip0 + q/k weights.
            # Everything else (strips 1-3, wv/bv/wo) is emitted later so the
            # PE's initial dependency barrier doesn't wait on those transfers.
            emit_strip_dma(0, split=True)
            wq_sb = cpool.tile([P, N_DC, ES], DT_QK)
            wk_sb = cpool.tile([P, N_DC, ES], DT_QK)
            nc.scalar.dma_start(wq_sb[:], wqT_d[:])
            nc.scalar.dma_start(wk_sb[:], wkT_d[:])
            bq_sb = cpool.tile([ES, 1], F32)
            bk_sb = cpool.tile([ES, 1], F32)
            nc.scalar.dma_start(bq_sb[:], bq_d[:])
            nc.scalar.dma_start(bk_sb[:], bk_d[:])

            ones_row = cpool.tile([1, ES], F32)
            nc.vector.memset(ones_row[:], 1.0)

            # ---- persistent activations ---------------------------------
            qT_sb = qkv_pool.tile([P, BS], DT_QK)     # [feat 128, s 4096]
            kT_sb = qkv_pool.tile([P, BS], DT_QK)
            # per-head V, interleaved, each with a ones column (rowsum rides PV)
            vAB_sb = qkv_pool.tile([P, N_CH, 2, HD + 1], DT_ATT)
            oT_sb = qkv_pool.tile([P, BS], DT_OUT)    # normalized attn out, [feat, s]
            nc.vector.memset(vAB_sb[:, :, :, HD : HD + 1], 1.0)

            # ---- emission helpers ---------------------------------------
            def emit_qk_piece(sb, which):
                s0 = sb * NF
                strip = strips[sb]
                w_sb, bias, dst = ((wq_sb, bq_sb, qT_sb) if which == "q"
                                   else (wk_sb, bk_sb, kT_sb))
                p_ps = ps.tile([P, NF], F32, tag="misc", bufs=2, name=f"{which}{sb}_ps")
                for j in range(N_DC):
                    nc.tensor.matmul(p_ps[:], w_sb[:, j], strip[:, j],
                                     start=(j == 0), stop=(j == N_DC - 1))
                nc.vector.tensor_scalar_add(dst[:, s0 : s0 + NF], p_ps[:], bias[:])

            def emit_v_piece(sb, ss):
                strip = strips[sb]
                ch = sb * (NF // P) + ss
                v_ps = ps.tile([P, ES], F32, tag="misc", bufs=2, name=f"v{ch}_ps")
                for j in range(N_DC):
                    nc.tensor.matmul(v_ps[:], strip[:, j, ss * P : (ss + 1) * P],
                                     wv_sb[:, j],
                                     start=(j == 0), stop=(j == N_DC - 1))
                # single strided eviction: [128,(2,64)] -> the two head slots
                nc.vector.tensor_add(
                    vAB_sb[:, ch, :, 0:HD],
                    v_ps[:].rearrange("p (h e) -> p h e", h=2),
                    bv_bc[:].rearrange("p (h e) -> p h e", h=2))

            y_queue = []
            n_yev = [0]

            def emit_recip_ln(oraw, q0):
                # oraw: [65, 2, 512] fp32 SBUF; row 64 = both heads' rowsums.
                # 1/rowsum as exp(-ln(rowsum)) on ACT (2 ULP; the DVE
                # reciprocal takes 3.3us/call), one wide 1024-col pass per
                # step -- emitted in separate slots so ACT never bursts --
                # then ONE partition-broadcast for both heads on GPSIMD.
                lg2 = pt_pool.tile([1, 2, NF], F32, tag="lg", bufs=4)
                nc.scalar.activation(lg2[:], oraw[HD : HD + 1],
                                     mybir.ActivationFunctionType.Ln)
                return (oraw, lg2, q0)

            def emit_recip_exp(oraw, lg2, q0):
                rcp2 = pt_pool.tile([1, 2, NF], F32, tag="rcp", bufs=4)
                nc.scalar.activation(rcp2[:], lg2[:],
                                     mybir.ActivationFunctionType.Exp,
                                     scale=-1.0)
                bc2 = pt_pool.tile([HD, 2, NF], F32, tag="bc", bufs=3)
                nc.gpsimd.partition_broadcast(bc2[:], rcp2[:])
                return (oraw, bc2, q0)

            def emit_recip_chain(oraw, q0):
                return emit_recip_exp(*emit_recip_ln(oraw, q0))

            def emit_apply(oraw, bc2, q0, on_dve=True):
                eng = nc.vector if on_dve else nc.gpsimd
                for hidx, part in ((0, 0), (1, HD)):
                    eng.tensor_mul(
                        oT_sb[part : part + HD, q0 : q0 + NF],
                        oraw[0:HD, hidx], bc2[:, hidx])
                for ss in range(NF // P):
                    for ec in range(D // NF):
                        y_queue.append((q0 + ss * P, ec))

            def emit_yproj(s0, ec, evict=None, queues=(nc.sync, nc.gpsimd)):
                y_ps = ps.tile([P, NF], F32, tag="misc", bufs=2)
                nc.tensor.matmul(y_ps[:], oT_sb[:, s0 : s0 + P],
                                 wo_sb[:, ec * NF : (ec + 1) * NF],
                                 start=True, stop=True)
                y_sb = y_pool.tile([P, NF], DT_Y, tag="y")
                # alternate the PSUM->SBUF eviction between DVE and ACT to
                # keep both below saturation (tail overrides via `evict`)
                if evict is None:
                    evict = "act" if n_yev[0] % 4 == 3 else "dve"
                if evict == "dve":
                    nc.vector.tensor_copy(y_sb[:], y_ps[:])
                else:
                    nc.scalar.activation(y_sb[:], y_ps[:],
                                         mybir.ActivationFunctionType.Copy)
                # split each tile's output DMA by partition-half across
                # both rings: halves per-tile transfer latency (keeps full
                # 1KB per-partition lines) and loads the rings evenly
                cs = slice(ec * NF, (ec + 1) * NF)
                qa = queues[n_yev[0] % len(queues)]
                qb = queues[(n_yev[0] + 1) % len(queues)]
                qa.dma_start(y_d[s0 : s0 + P // 2, cs], y_sb[0 : P // 2, :])
                qb.dma_start(y_d[s0 + P // 2 : s0 + P, cs], y_sb[P // 2 :, :])
                n_yev[0] += 1

            # ---- projections for batch 0 (k/v first; q trails as filler) -
            # k0/q0 first (they only need strip0 + wq/wk); the remaining
            # input DMAs issue while the PE chews on them.
            emit_qk_piece(0, "k")
            emit_qk_piece(0, "q")
            wv_sb = cpool.tile([P, N_DC, ES], DT_V)
            nc.scalar.dma_start(wv_sb[:], wvT_d[:])
            bv_row = cpool.tile([1, ES], F32)
            nc.scalar.dma_start(bv_row[:], bv_d[:])
            emit_strip_dma(1)
            wo_sb = cpool.tile([ES, D], DT_OUT)
            nc.scalar.dma_start(wo_sb[:], woT_d[:])

            # bv broadcast to all 128 partitions via rank-1 matmul
            bv_bc_ps = ps.tile([P, ES], F32, tag="misc", bufs=2)
            nc.tensor.matmul(bv_bc_ps[:], ones_row[:], bv_row[:], start=True, stop=True)
            bv_bc = cpool.tile([P, ES], F32)
            nc.vector.tensor_copy(bv_bc[:], bv_bc_ps[:])

            for ss in range(NF // P):
                emit_v_piece(0, ss)
            for sb in range(1, N_SB // 2):
                if sb + 1 < N_SB // 2:
                    emit_strip_dma(sb + 1)
                emit_qk_piece(sb, "k")
                for ss in range(NF // P):
                    emit_v_piece(sb, ss)

            # filler work queues: remaining q pieces + batch-1 projections
            # drip-feed into batch-0 attention; deferred output projections
            # drip into batch-1. q_sb{i} must complete before (b0, qc=i).
            a_queue = [("q", 1), ("q", 2), ("q", 3)]
            for sb in range(N_SB // 2, N_SB):
                a_queue.append(("dma", sb))
                a_queue.append(("q", sb))
                a_queue.append(("k", sb))
                for ss in range(NF // P):
                    a_queue.append(("v", sb, ss))

            def emit_a_piece():
                piece = a_queue.pop(0)
                if piece[0] == "dma":
                    emit_strip_dma(piece[1])
                    if a_queue:
                        emit_a_piece()  # dma is async; also emit a compute piece
                elif piece[0] in ("q", "k"):
                    emit_qk_piece(piece[1], piece[0])
                else:
                    emit_v_piece(piece[1], piece[2])

            # ---- attention: one continuous software pipeline -------------
            # Global stream over 64 ST pair-slots (8 per (b,qc) iteration);
            # PV consumption lags ST/exp by one pair and crosses iteration
            # boundaries, so the PE pipeline never drains mid-kernel.
            n_iters = B * N_QC
            total_pairs = n_iters * N_KP
            o_tiles = {}
            ptq = {}
            pending = None
            ln_state = None
            norm_state = None

            for g in range(total_pairs + 2):
                if g < total_pairs:
                    it = g // N_KP
                    kp = g % N_KP
                    b, qc = it // N_QC, it % N_QC
                    if kp == 0 and b == 1 and qc == 0:
                        while a_queue:
                            emit_a_piece()
                    q0 = b * S + qc * NF
                    # A0,A1 then B0,B1: head B's first-block weights are
                    # preloaded into row-groups 2-3 before A0 issues, so the
                    # B matmuls co-start with the A streams on groups 0-1.
                    st2A = ps.tile([P, 2, NF], F32, tag="st2", bufs=2)
                    st2B = ps.tile([P, 2, NF], F32, tag="st2", bufs=2)
                    k0p = b * S + kp * 2 * P
                    nc.tensor.ldweights(kT_sb[HD:P, k0p : k0p + P],
                                        tile_position=(64, 0))
                    for half in range(2):
                        k0 = b * S + (kp * 2 + half) * P
                        nc.tensor.matmul(st2A[:, half], kT_sb[0:HD, k0 : k0 + P],
                                         qT_sb[0:HD, q0 : q0 + NF],
                                         start=True, stop=True)
                    pt2A = pt_pool.tile([P, 2, NF], DT_ATT, tag="pt", bufs=14)
                    nc.scalar.activation(pt2A[:], st2A[:],
                                         mybir.ActivationFunctionType.Exp,
                                         scale=inv_sqrt_hd)
                    for half in range(2):
                        k0 = b * S + (kp * 2 + half) * P
                        nc.tensor.matmul(st2B[:, half], kT_sb[HD:P, k0 : k0 + P],
                                         qT_sb[HD:P, q0 : q0 + NF],
                                         start=True, stop=True)
                    pt2B = pt_pool.tile([P, 2, NF], DT_ATT, tag="pt", bufs=14)
                    # head A always on ACT; head B on DVE (Schraudolph)
                    # except at iteration boundaries (kp 0 and 7), where the
                    # DVE must service the o-copy/apply chain promptly.
                    if kp in (0, 1):
                        nc.scalar.activation(pt2B[:], st2B[:],
                                             mybir.ActivationFunctionType.Exp,
                                             scale=inv_sqrt_hd)
                    else:
                        nc.vector.tensor_scalar(pt2B[:].bitcast(U16), st2B[:],
                                                A16 * inv_sqrt_hd, B16,
                                                mybir.AluOpType.mult,
                                                mybir.AluOpType.add)
                    ptq[g] = (pt2A, pt2B)

                    # fillers ride the ST side of the stream
                    if b == 0:
                        if a_queue:
                            emit_a_piece()
                    else:
                        # keep ~6 y tiles in reserve: they run during the
                        # final iteration's normalization chain so the PE
                        # never idles >3.4us (which would re-throttle HAM)
                        # near the end, build a reserve of y tiles for the
                        # tail (they keep the PE warm through the final
                        # normalization chain); the last slots hold all
                        if g >= total_pairs - 4:
                            thresh = 99
                        elif g >= total_pairs - 16:
                            thresh = 8
                        else:
                            thresh = 0
                        for _ in range(2):
                            if len(y_queue) > thresh:
                                emit_yproj(*y_queue.pop(0))
                    if kp == 2 and pending is not None:
                        ln_state = emit_recip_ln(*pending)
                        pending = None
                    if kp == 3 and ln_state is not None:
                        norm_state = emit_recip_exp(*ln_state)
                        ln_state = None
                    if kp == 5 and norm_state is not None:
                        emit_apply(*norm_state)
                        norm_state = None

                # PV consumption lags the ST/exp stream by TWO slots so the
                # Schraudolph exp (queued on the DVE behind evictions) always
                # completes before its PV matmuls need it.
                if g >= 2:
                    pg = g - 2
                    it = pg // N_KP
                    kp = pg % N_KP
                    b, qc = it // N_QC, it % N_QC
                    q0 = b * S + qc * NF
                    if kp == 0:
                        o_tiles[it] = ps.tile([HD + 1, 2, NF], F32, tag="o",
                                              bufs=1, name=f"o2_{it}")
                    o2_ps = o_tiles[it]
                    pt2A, pt2B = ptq.pop(pg)
                    for half in range(2):
                        kb = kp * 2 + half
                        gkb = b * N_KB + kb
                        nc.tensor.matmul(o2_ps[:, 0], vAB_sb[:, gkb, 0], pt2A[:, half],
                                         start=(kb == 0), stop=(kb == N_KB - 1))
                        nc.tensor.matmul(o2_ps[:, 1], vAB_sb[:, gkb, 1], pt2B[:, half],
                                         start=(kb == 0), stop=(kb == N_KB - 1))
                    if kp == N_KP - 1:
                        # iteration finished: copy both heads' raw o (incl.
                        # the rowsum row) out in one wide pass, freeing the
                        # o banks; normalization is deferred. The copy runs
                        # on ACT (DVE carries the Schraudolph exps), except
                        # for the last iteration where ACT is about to run
                        # the recip chain and DVE shortens the tail path.
                        oraw = pt_pool.tile([HD + 1, 2, NF], F32, tag="oraw", bufs=3)
                        nc.vector.tensor_copy(oraw[:], o2_ps[:])
                        del o_tiles[it]
                        pending = (oraw, q0)
                        if it == n_iters - 1:
                            final_o_ps = o2_ps

            # ---- tail: the reserved y tiles keep the PE warm while the
            # final iteration's normalization chain runs; the last 8 tiles
            # then flow with evictions alternating across DVE/ACT.
            reserved = list(y_queue)
            y_queue.clear()
            tailq = (nc.sync, nc.gpsimd)
            oraw_f, q0_f = pending
            bcs = []
            for hidx in (0, 1):
                # Ln reads the rowsum row straight from PSUM, so the recip
                # chain starts in parallel with the DVE oraw copy; per-head
                # chains let head A's broadcast begin ~1.4us earlier
                lg = pt_pool.tile([1, NF], F32, tag="lg", bufs=4, name=f"flg{hidx}")
                rcp = pt_pool.tile([1, NF], F32, tag="rcp", bufs=4, name=f"frcp{hidx}")
                nc.scalar.activation(lg[:], final_o_ps[HD : HD + 1, hidx],
                                     mybir.ActivationFunctionType.Ln)
                nc.scalar.activation(rcp[:], lg[:],
                                     mybir.ActivationFunctionType.Exp,
                                     scale=-1.0)
                bc = pt_pool.tile([HD, NF], F32, tag="bc", bufs=3, name=f"fbc{hidx}")
                nc.gpsimd.partition_broadcast(bc[:], rcp[:])
                bcs.append(bc)
            for i, (s0, ec) in enumerate(reserved):
                emit_yproj(s0, ec, evict="dve" if i < 5 else "act", queues=tailq)
            for hidx, part in ((0, 0), (1, HD)):
                nc.vector.tensor_mul(oT_sb[part : part + HD, q0_f : q0_f + NF],
                                     oraw_f[0:HD, hidx], bcs[hidx][:])
            final8 = [(q0_f + ss * P, ec)
                      for ss in range(NF // P) for ec in range(D // NF)]
            for i, (s0, ec) in enumerate(final8):
                emit_yproj(s0, ec, evict="dve" if i % 2 == 0 else "act",
                           queues=tailq)

    nc.compile()
    return nc


def kernel(x, Wq, bq, Wk, bk, Wv, bv, Wo, bo, _trace=False):
    global last_results
    x = np.asarray(x, dtype=np.float32)
    Wq, bq = np.asarray(Wq, np.float32), np.asarray(bq, np.float32)
    Wk, bk = np.asarray(Wk, np.float32), np.asarray(bk, np.float32)
    Wv, bv = np.asarray(Wv, np.float32), np.asarray(bv, np.float32)
    Wo, bo = np.asarray(Wo, np.float32), np.asarray(bo, np.float32)

    if "nc" not in _cache:
        _cache["nc"] = _build()
    nc = _cache["nc"]

    dt_qk, dt_v, dt_out = _np_dt(DT_QK), _np_dt(DT_V), _np_dt(DT_OUT)
    xT = np.ascontiguousarray(x.reshape(BS, D).T)
    xT_qk = xT.astype(dt_qk, copy=False)

    def _w_sbuf(WT):  # [D, ES] -> SBUF layout [P, N_DC, ES]
        return np.ascontiguousarray(
            WT.reshape(N_DC, P, ES).transpose(1, 0, 2))

    in_maps = []
    for c in range(NCORES):
        sl = slice(c * ES, (c + 1) * ES)
        in_maps.append({
            "xT": xT_qk,
            "wqT": _w_sbuf(Wq[sl].T.astype(dt_qk)),
            "wkT": _w_sbuf(Wk[sl].T.astype(dt_qk)),
            "wvT": _w_sbuf(Wv[sl].T.astype(dt_v)),
            "bq": np.ascontiguousarray(bq[sl, None]),
            "bk": np.ascontiguousarray(bk[sl, None]),
            "bv": np.ascontiguousarray(bv[None, sl]),
            "woT": np.ascontiguousarray(Wo[:, sl].T).astype(dt_out, copy=False),
        })

    res = bass_utils.run_bass_kernel_spmd(
        nc, in_maps, core_ids=list(range(NCORES)), trace=_trace)
    last_results = res

    y = res.results[0]["y"].astype(np.float64)
    for c in range(1, NCORES):
        y += res.results[c]["y"].astype(np.float64)
    y = (y + bo).astype(np.float32)
    return y.reshape(B, S, D)


# revision 52
# speedup vs baseline: 1.0046x; 1.0046x over previous
"""Chunked (= full, non-causal) multi-head self-attention on 8 TRN2 NeuronCores.

Problem: B=2, S=2048, D=1024, H=16 heads (head_dim 64), torch-Linear-style
projections (y = x @ W.T + b), softmax attention, output projection.

Sharding: head-parallel. Core c owns heads {2c, 2c+1} = feature slice
[128c, 128c+128). Each core computes q/k/v for its slice from the full x
(replicated), runs attention for its 4 (batch, head) pairs, and produces a
partial output projection with its 128-row slice of Wo. Host sums the 8
partials and adds bo.

Layout: scores are computed transposed, ST[k, q] (keys on partitions), so the
softmax exp output PT feeds the P@V matmul directly (contraction over k on
partitions) with no on-chip transposes anywhere — x and the weights are
pre-transposed on the host. The two heads' K=64 score matmuls land on PE
row-groups 0-1/2-3. The softmax denominator rides as row 64 of the PV output
via a ones-column appended to V (M=65); both heads' PV accumulators share one
2-bank PSUM tile so the iteration epilogue (raw-o copy, reciprocal via
exp(-ln) on ACT, GPSIMD partition-broadcast, normalize-multiply) runs as wide
1024-col ops, deferred one iteration so it never stalls the PE.

Softmax exp is split across two engines: the ACT (scalar) engine computes the
table-based exp for ~1/3 of the score tiles, and the DVE (vector) engine
computes the rest with a one-instruction Schraudolph exp — bf16 bits =
u16(x * 128*log2(e)/sqrt(hd) + (127*128 - 5.51)), i.e. a tensor_scalar mul+add
with u16-saturating convert whose output is bitcast to bf16 (underflow
saturates to +0.0 exactly). The per-element error (~±3%, ~1.8% mean) averages
out below 2e-3 in the softmax-weighted sum, and each head-tile's denominator
rides the same approximated values, so normalization stays self-consistent.

Scheduling: one software-pipelined stream keeps the PE densely busy (TRN2's
HAM clock-gate halves the PE clock after ~3.4us of idle): x-strip DMAs issue
on the Sync queue ahead of the weight DMAs (Scalar queue), batch-0 k/v (+q0)
projections run first, then batch-0 attention interleaved with the remaining
projections, then batch-1 attention interleaved with the deferred output
projections. y tiles evict PSUM->SBUF alternating between DVE and ACT and
stream out as bf16 (host accumulates in fp64).

Precision: everything bf16 in, fp32 accumulate, bf16 partial outputs
(~6e-3 rel err overall).
"""

import sys

if "/opt/trn_rl_repo" not in sys.path:
    sys.path.insert(0, "/opt/trn_rl_repo")

import numpy as np

import concourse.bacc as bacc
import concourse.mybir as mybir
import concourse.tile as tile
from concourse import bass_utils

# Route Exp to the activation-table set that also holds Ln, so the softmax
# exps and the reciprocal-via-exp(-ln(x)) trick share one table (the default
# per-function choice would ping-pong table loads at ~2.7us each).
_orig_get_activation_tables = bacc.get_activation_tables


def _patched_get_activation_tables(arch):
    out = {}
    for name, funcs in dict(_orig_get_activation_tables(arch)).items():
        if name != "natural_log_exp_and_others":
            funcs = {f for f in funcs if f != mybir.ActivationFunctionType.Exp}
        out[name] = funcs
    return out


bacc.get_activation_tables = _patched_get_activation_tables

B, S, D, H = 2, 2048, 1024, 16
HD = D // H          # 64
NCORES = 8
ES = D // NCORES     # 128 features (= 2 heads) per core
BS = B * S           # 4096 rows total

P = 128              # partitions
NF = 512             # matmul free-dim tile
N_SB = BS // NF      # 8 s-blocks of 512
N_DC = D // P        # 8 contraction chunks of 128
N_KB = S // P        # 16 key blocks of 128 per batch
N_KP = N_KB // 2     # 8 key-block PAIRS per batch
N_QC = S // NF       # 4 query chunks of 512 per batch
N_CH = BS // P       # 32 global 128-row chunks

F32 = mybir.dt.float32
BF16 = mybir.dt.bfloat16
U16 = mybir.dt.uint16

DT_QK = BF16         # x/Wq/Wk inputs for q,k projections + score matmuls
DT_V = BF16          # x/Wv inputs for v projection
DT_ATT = BF16        # attention weights (exp output) and V in the P@V matmul
DT_OUT = BF16        # output projection inputs (OT, Wo)
DT_Y = BF16          # y partials shipped to the host

# Schraudolph-to-bf16 exp constants: bf16 bits = u16(x*A16 + B16)
LOG2E = float(np.log2(np.e))
A16 = LOG2E * 128.0
B16 = 127.0 * 128.0 - 128.0 * 0.04305

_cache = {}
last_results = None          # test.py reads exec_time_ns off this


def _np_dt(dt):
    import ml_dtypes

    return np.dtype(ml_dtypes.bfloat16) if dt == mybir.dt.bfloat16 else np.dtype(np.float32)


def _build():
    nc = bacc.Bacc("TRN2", target_bir_lowering=False, debug=False)

    xT_d = nc.dram_tensor("xT", [D, BS], DT_QK, kind="ExternalInput")
    # weights arrive host-pre-rearranged to the SBUF layout [P, N_DC, ES]
    # (dense per-partition rows -> trivial DMA descriptors, ~5x faster issue)
    wqT_d = nc.dram_tensor("wqT", [P, N_DC, ES], DT_QK, kind="ExternalInput")
    wkT_d = nc.dram_tensor("wkT", [P, N_DC, ES], DT_QK, kind="ExternalInput")
    wvT_d = nc.dram_tensor("wvT", [P, N_DC, ES], DT_V, kind="ExternalInput")
    bq_d = nc.dram_tensor("bq", [ES, 1], F32, kind="ExternalInput")
    bk_d = nc.dram_tensor("bk", [ES, 1], F32, kind="ExternalInput")
    bv_d = nc.dram_tensor("bv", [1, ES], F32, kind="ExternalInput")
    woT_d = nc.dram_tensor("woT", [ES, D], DT_OUT, kind="ExternalInput")
    y_d = nc.dram_tensor("y", [BS, D], DT_Y, kind="ExternalOutput")

    inv_sqrt_hd = 1.0 / float(np.sqrt(HD))

    with tile.TileContext(nc) as tc:
        with tc.tile_pool(name="const", bufs=1) as cpool, \
             tc.tile_pool(name="xt", bufs=3) as xt_pool, \
             tc.tile_pool(name="qkv", bufs=1) as qkv_pool, \
             tc.tile_pool(name="pt", bufs=14) as pt_pool, \
             tc.tile_pool(name="ysb", bufs=6) as y_pool, \
             tc.tile_pool(name="ps", bufs=1, space="PSUM") as ps:

            # ---- input DMAs: x strips on Sync, weights on Scalar ---------
            strips = {}

            xT_r = xT_d.ap().rearrange("(a p) s -> p a s", p=P)

            def emit_strip_dma(sb, split=False):
                strip = xt_pool.tile([P, N_DC, NF], DT_QK, tag="strip", name=f"strip{sb}")
                if split:
                    # two half-DMAs: the first qk matmuls (contraction chunks
                    # 0-3) start after only half the strip has landed
                    sl = xT_r[:, :, sb * NF : (sb + 1) * NF]
                    nc.sync.dma_start(strip[:, 0 : N_DC // 2], sl[:, 0 : N_DC // 2])
                    nc.sync.dma_start(strip[:, N_DC // 2 :], sl[:, N_DC // 2 :])
                else:
                    nc.sync.dma_start(strip[:], xT_r[:, :, sb * NF : (sb + 1) * NF])
                strips[sb] = strip

            # minimal DMA set before the first matmul: strip0 + q/k weights.
            # Everything else (strips 1-3, wv/bv/wo) is emitted later so the
            # PE's initial dependency barrier doesn't wait on those transfers.
            emit_strip_dma(0, split=True)
            wq_sb = cpool.tile([P, N_DC, ES], DT_QK)
            wk_sb = cpool.tile([P, N_DC, ES], DT_QK)
            nc.scalar.dma_start(wq_sb[:], wqT_d[:])
            nc.scalar.dma_start(wk_sb[:], wkT_d[:])
            bq_sb = cpool.tile([ES, 1], F32)
            bk_sb = cpool.tile([ES, 1], F32)
            nc.scalar.dma_start(bq_sb[:], bq_d[:])
            nc.scalar.dma_start(bk_sb[:], bk_d[:])

            ones_row = cpool.tile([1, ES], F32)
            nc.vector.memset(ones_row[:], 1.0)

            # ---- persistent activations ---------------------------------
            qT_sb = qkv_pool.tile([P, BS], DT_QK)     # [feat 128, s 4096]
            kT_sb = qkv_pool.tile([P, BS], DT_QK)
            # per-head V, interleaved, each with a ones column (rowsum rides PV)
            vAB_sb = qkv_pool.tile([P, N_CH, 2, HD + 1], DT_ATT)
            oT_sb = qkv_pool.tile([P, BS], DT_OUT)    # normalized attn out, [feat, s]
            nc.vector.memset(vAB_sb[:, :, :, HD : HD + 1], 1.0)

            # ---- emission helpers ---------------------------------------
            def emit_qk_piece(sb, which):
                s0 = sb * NF
                strip = strips[sb]
                w_sb, bias, dst = ((wq_sb, bq_sb, qT_sb) if which == "q"
                                   else (wk_sb, bk_sb, kT_sb))
                p_ps = ps.tile([P, NF], F32, tag="misc", bufs=2, name=f"{which}{sb}_ps")
                for j in range(N_DC):
                    nc.tensor.matmul(p_ps[:], w_sb[:, j], strip[:, j],
                                     start=(j == 0), stop=(j == N_DC - 1))
                nc.vector.tensor_scalar_add(dst[:, s0 : s0 + NF], p_ps[:], bias[:])

            def emit_v_piece(sb, ss):
                strip = strips[sb]
                ch = sb * (NF // P) + ss
                v_ps = ps.tile([P, ES], F32, tag="misc", bufs=2, name=f"v{ch}_ps")
                for j in range(N_DC):
                    nc.tensor.matmul(v_ps[:], strip[:, j, ss * P : (ss + 1) * P],
                                     wv_sb[:, j],
                                     start=(j == 0), stop=(j == N_DC - 1))
                # single strided eviction: [128,(2,64)] -> the two head slots
                nc.vector.tensor_add(
                    vAB_sb[:, ch, :, 0:HD],
                    v_ps[:].rearrange("p (h e) -> p h e", h=2),
                    bv_bc[:].rearrange("p (h e) -> p h e", h=2))

            y_queue = []
            n_yev = [0]

            def emit_recip_ln(oraw, q0):
                # oraw: [65, 2, 512] fp32 SBUF; row 64 = both heads' rowsums.
                # 1/rowsum as exp(-ln(rowsum)) on ACT (2 ULP; the DVE
                # reciprocal takes 3.3us/call), one wide 1024-col pass per
                # step -- emitted in separate slots so ACT never bursts --
                # then ONE partition-broadcast for both heads on GPSIMD.
                lg2 = pt_pool.tile([1, 2, NF], F32, tag="lg", bufs=4)
                nc.scalar.activation(lg2[:], oraw[HD : HD + 1],
                                     mybir.ActivationFunctionType.Ln)
                return (oraw, lg2, q0)

            def emit_recip_exp(oraw, lg2, q0):
                rcp2 = pt_pool.tile([1, 2, NF], F32, tag="rcp", bufs=4)
                nc.scalar.activation(rcp2[:], lg2[:],
                                     mybir.ActivationFunctionType.Exp,
                                     scale=-1.0)
                bc2 = pt_pool.tile([HD, 2, NF], F32, tag="bc", bufs=3)
                nc.gpsimd.partition_broadcast(bc2[:], rcp2[:])
                return (oraw, bc2, q0)

            def emit_recip_chain(oraw, q0):
                return emit_recip_exp(*emit_recip_ln(oraw, q0))

            def emit_apply(oraw, bc2, q0, on_dve=True):
                eng = nc.vector if on_dve else nc.gpsimd
                for hidx, part in ((0, 0), (1, HD)):
                    eng.tensor_mul(
                        oT_sb[part : part + HD, q0 : q0 + NF],
                        oraw[0:HD, hidx], bc2[:, hidx])
                for ss in range(NF // P):
                    for ec in range(D // NF):
                        y_queue.append((q0 + ss * P, ec))

            def emit_yproj(s0, ec, evict=None, queues=(nc.sync, nc.gpsimd)):
                y_ps = ps.tile([P, NF], F32, tag="misc", bufs=2)
                nc.tensor.matmul(y_ps[:], oT_sb[:, s0 : s0 + P],
                                 wo_sb[:, ec * NF : (ec + 1) * NF],
                                 start=True, stop=True)
                y_sb = y_pool.tile([P, NF], DT_Y, tag="y")
                # alternate the PSUM->SBUF eviction between DVE and ACT to
                # keep both below saturation (tail overrides via `evict`)
                if evict is None:
                    evict = "act" if n_yev[0] % 4 == 3 else "dve"
                if evict == "dve":
                    nc.vector.tensor_copy(y_sb[:], y_ps[:])
                else:
                    nc.scalar.activation(y_sb[:], y_ps[:],
                                         mybir.ActivationFunctionType.Copy)
                # round-robin the output DMA across rings so the drain
                # isn't serialized on one queue
                q = queues[n_yev[0] % len(queues)]
                q.dma_start(y_d[s0 : s0 + P, ec * NF : (ec + 1) * NF], y_sb[:])
                n_yev[0] += 1

            # ---- projections for batch 0 (k/v first; q trails as filler) -
            # k0/q0 first (they only need strip0 + wq/wk); the remaining
            # input DMAs issue while the PE chews on them.
            emit_qk_piece(0, "k")
            emit_qk_piece(0, "q")
            wv_sb = cpool.tile([P, N_DC, ES], DT_V)
            nc.scalar.dma_start(wv_sb[:], wvT_d[:])
            bv_row = cpool.tile([1, ES], F32)
            nc.scalar.dma_start(bv_row[:], bv_d[:])
            emit_strip_dma(1)
            wo_sb = cpool.tile([ES, D], DT_OUT)
            nc.scalar.dma_start(wo_sb[:], woT_d[:])

            # bv broadcast to all 128 partitions via rank-1 matmul
            bv_bc_ps = ps.tile([P, ES], F32, tag="misc", bufs=2)
            nc.tensor.matmul(bv_bc_ps[:], ones_row[:], bv_row[:], start=True, stop=True)
            bv_bc = cpool.tile([P, ES], F32)
            nc.vector.tensor_copy(bv_bc[:], bv_bc_ps[:])

            for ss in range(NF // P):
                emit_v_piece(0, ss)
            for sb in range(1, N_SB // 2):
                if sb + 1 < N_SB // 2:
                    emit_strip_dma(sb + 1)
                emit_qk_piece(sb, "k")
                for ss in range(NF // P):
                    emit_v_piece(sb, ss)

            # filler work queues: remaining q pieces + batch-1 projections
            # drip-feed into batch-0 attention; deferred output projections
            # drip into batch-1. q_sb{i} must complete before (b0, qc=i).
            a_queue = [("q", 1), ("q", 2), ("q", 3)]
            for sb in range(N_SB // 2, N_SB):
                a_queue.append(("dma", sb))
                a_queue.append(("q", sb))
                a_queue.append(("k", sb))
                for ss in range(NF // P):
                    a_queue.append(("v", sb, ss))

            def emit_a_piece():
                piece = a_queue.pop(0)
                if piece[0] == "dma":
                    emit_strip_dma(piece[1])
                    if a_queue:
                        emit_a_piece()  # dma is async; also emit a compute piece
                elif piece[0] in ("q", "k"):
                    emit_qk_piece(piece[1], piece[0])
                else:
                    emit_v_piece(piece[1], piece[2])

            # ---- attention: one continuous software pipeline -------------
            # Global stream over 64 ST pair-slots (8 per (b,qc) iteration);
            # PV consumption lags ST/exp by one pair and crosses iteration
            # boundaries, so the PE pipeline never drains mid-kernel.
            n_iters = B * N_QC
            total_pairs = n_iters * N_KP
            o_tiles = {}
            ptq = {}
            pending = None
            ln_state = None
            norm_state = None

            for g in range(total_pairs + 2):
                if g < total_pairs:
                    it = g // N_KP
                    kp = g % N_KP
                    b, qc = it // N_QC, it % N_QC
                    if kp == 0 and b == 1 and qc == 0:
                        while a_queue:
                            emit_a_piece()
                    q0 = b * S + qc * NF
                    # A0,A1 then B0,B1: head B's first-block weights are
                    # preloaded into row-groups 2-3 before A0 issues, so the
                    # B matmuls co-start with the A streams on groups 0-1.
                    st2A = ps.tile([P, 2, NF], F32, tag="st2", bufs=2)
                    st2B = ps.tile([P, 2, NF], F32, tag="st2", bufs=2)
                    k0p = b * S + kp * 2 * P
                    nc.tensor.ldweights(kT_sb[HD:P, k0p : k0p + P],
                                        tile_position=(64, 0))
                    for half in range(2):
                        k0 = b * S + (kp * 2 + half) * P
                        nc.tensor.matmul(st2A[:, half], kT_sb[0:HD, k0 : k0 + P],
                                         qT_sb[0:HD, q0 : q0 + NF],
                                         start=True, stop=True)
                    pt2A = pt_pool.tile([P, 2, NF], DT_ATT, tag="pt", bufs=14)
                    nc.scalar.activation(pt2A[:], st2A[:],
                                         mybir.ActivationFunctionType.Exp,
                                         scale=inv_sqrt_hd)
                    for half in range(2):
                        k0 = b * S + (kp * 2 + half) * P
                        nc.tensor.matmul(st2B[:, half], kT_sb[HD:P, k0 : k0 + P],
                                         qT_sb[HD:P, q0 : q0 + NF],
                                         start=True, stop=True)
                    pt2B = pt_pool.tile([P, 2, NF], DT_ATT, tag="pt", bufs=14)
                    # head A always on ACT; head B on DVE (Schraudolph)
                    # except at iteration boundaries (kp 0 and 7), where the
                    # DVE must service the o-copy/apply chain promptly.
                    if kp in (0, 1):
                        nc.scalar.activation(pt2B[:], st2B[:],
                                             mybir.ActivationFunctionType.Exp,
                                             scale=inv_sqrt_hd)
                    else:
                        nc.vector.tensor_scalar(pt2B[:].bitcast(U16), st2B[:],
                                                A16 * inv_sqrt_hd, B16,
                                                mybir.AluOpType.mult,
                                                mybir.AluOpType.add)
                    ptq[g] = (pt2A, pt2B)

                    # fillers ride the ST side of the stream
                    if b == 0:
                        if a_queue:
                            emit_a_piece()
                    else:
                        # keep ~6 y tiles in reserve: they run during the
                        # final iteration's normalization chain so the PE
                        # never idles >3.4us (which would re-throttle HAM)
                        # near the end, build a reserve of y tiles for the
                        # tail (they keep the PE warm through the final
                        # normalization chain); the last slots hold all
                        if g >= total_pairs - 4:
                            thresh = 99
                        elif g >= total_pairs - 16:
                            thresh = 8
                        else:
                            thresh = 0
                        for _ in range(2):
                            if len(y_queue) > thresh:
                                emit_yproj(*y_queue.pop(0))
                    if kp == 2 and pending is not None:
                        ln_state = emit_recip_ln(*pending)
                        pending = None
                    if kp == 3 and ln_state is not None:
                        norm_state = emit_recip_exp(*ln_state)
                        ln_state = None
                    if kp == 5 and norm_state is not None:
                        emit_apply(*norm_state)
                        norm_state = None

                # PV consumption lags the ST/exp stream by TWO slots so the
                # Schraudolph exp (queued on the DVE behind evictions) always
                # completes before its PV matmuls need it.
                if g >= 2:
                    pg = g - 2
                    it = pg // N_KP
                    kp = pg % N_KP
                    b, qc = it // N_QC, it % N_QC
                    q0 = b * S + qc * NF
                    if kp == 0:
                        o_tiles[it] = ps.tile([HD + 1, 2, NF], F32, tag="o",
                                              bufs=1, name=f"o2_{it}")
                    o2_ps = o_tiles[it]
                    pt2A, pt2B = ptq.pop(pg)
                    for half in range(2):
                        kb = kp * 2 + half
                        gkb = b * N_KB + kb
                        nc.tensor.matmul(o2_ps[:, 0], vAB_sb[:, gkb, 0], pt2A[:, half],
                                         start=(kb == 0), stop=(kb == N_KB - 1))
                        nc.tensor.matmul(o2_ps[:, 1], vAB_sb[:, gkb, 1], pt2B[:, half],
                                         start=(kb == 0), stop=(kb == N_KB - 1))
                    if kp == N_KP - 1:
                        # iteration finished: copy both heads' raw o (incl.
                        # the rowsum row) out in one wide pass, freeing the
                        # o banks; normalization is deferred. The copy runs
                        # on ACT (DVE carries the Schraudolph exps), except
                        # for the last iteration where ACT is about to run
                        # the recip chain and DVE shortens the tail path.
                        oraw = pt_pool.tile([HD + 1, 2, NF], F32, tag="oraw", bufs=3)
                        nc.vector.tensor_copy(oraw[:], o2_ps[:])
                        del o_tiles[it]
                        pending = (oraw, q0)
                        if it == n_iters - 1:
                            final_o_ps = o2_ps

            # ---- tail: the reserved y tiles keep the PE warm while the
            # final iteration's normalization chain runs; the last 8 tiles
            # then flow with evictions alternating across DVE/ACT.
            reserved = list(y_queue)
            y_queue.clear()
            tailq = (nc.sync, nc.gpsimd)
            oraw_f, q0_f = pending
            bcs = []
            for hidx in (0, 1):
                # Ln reads the rowsum row straight from PSUM, so the recip
                # chain starts in parallel with the DVE oraw copy; per-head
                # chains let head A's broadcast begin ~1.4us earlier
                lg = pt_pool.tile([1, NF], F32, tag="lg", bufs=4, name=f"flg{hidx}")
                rcp = pt_pool.tile([1, NF], F32, tag="rcp", bufs=4, name=f"frcp{hidx}")
                nc.scalar.activation(lg[:], final_o_ps[HD : HD + 1, hidx],
                                     mybir.ActivationFunctionType.Ln)
                nc.scalar.activation(rcp[:], lg[:],
                                     mybir.ActivationFunctionType.Exp,
                                     scale=-1.0)
                bc = pt_pool.tile([HD, NF], F32, tag="bc", bufs=3, name=f"fbc{hidx}")
                nc.gpsimd.partition_broadcast(bc[:], rcp[:])
                bcs.append(bc)
            for i, (s0, ec) in enumerate(reserved):
                emit_yproj(s0, ec, evict="dve" if i < 5 else "act", queues=tailq)
            for hidx, part in ((0, 0), (1, HD)):
                nc.vector.tensor_mul(oT_sb[part : part + HD, q0_f : q0_f + NF],
                                     oraw_f[0:HD, hidx], bcs[hidx][:])
            final8 = [(q0_f + ss * P, ec)
                      for ss in range(NF // P) for ec in range(D // NF)]
            for i, (s0, ec) in enumerate(final8):
                emit_yproj(s0, ec, evict="dve" if i % 2 == 0 else "act",
                           queues=tailq)

    nc.compile()
    return nc


def kernel(x, Wq, bq, Wk, bk, Wv, bv, Wo, bo, _trace=False):
    global last_results
    x = np.asarray(x, dtype=np.float32)
    Wq, bq = np.asarray(Wq, np.float32), np.asarray(bq, np.float32)
    Wk, bk = np.asarray(Wk, np.float32), np.asarray(bk, np.float32)
    Wv, bv = np.asarray(Wv, np.float32), np.asarray(bv, np.float32)
    Wo, bo = np.asarray(Wo, np.float32), np.asarray(bo, np.float32)

    if "nc" not in _cache:
        _cache["nc"] = _build()
    nc = _cache["nc"]

    dt_qk, dt_v, dt_out = _np_dt(DT_QK), _np_dt(DT_V), _np_dt(DT_OUT)
    xT = np.ascontiguousarray(x.reshape(BS, D).T)
    xT_qk = xT.astype(dt_qk, copy=False)

    def _w_sbuf(WT):  # [D, ES] -> SBUF layout [P, N_DC, ES]
        return np.ascontiguousarray(
            WT.reshape(N_DC, P, ES).transpose(1, 0, 2))

    in_maps = []
    for c in range(NCORES):
        sl = slice(c * ES, (c + 1) * ES)
        in_maps.append({
            "xT": xT_qk,
            "wqT": _w_sbuf(Wq[sl].T.astype(dt_qk)),
            "wkT": _w_sbuf(Wk[sl].T.astype(dt_qk)),
            "wvT": _w_sbuf(Wv[sl].T.astype(dt_v)),
            "bq": np.ascontiguousarray(bq[sl, None]),
            "bk": np.ascontiguousarray(bk[sl, None]),
            "bv": np.ascontiguousarray(bv[None, sl]),
            "woT": np.ascontiguousarray(Wo[:, sl].T).astype(dt_out, copy=False),
        })

    res = bass_utils.run_bass_kernel_spmd(
        nc, in_maps, core_ids=list(range(NCORES)), trace=_trace)
    last_results = res

    y = res.results[0]["y"].astype(np.float64)
    for c in range(1, NCORES):
        y += res.results[c]["y"].astype(np.float64)
    y = (y + bo).astype(np.float32)
    return y.reshape(B, S, D)
